# revision 13
# baseline (speedup 1.0000x reference)
"""Trainium2 Bass kernel for nn_CitationClassifier (pooling/ridge).

Strategy: pure data parallel over the batch dim (256 = 8 cores x 32).
All heavy tensors are cast to fp16 and re-laid-out on the host into
partition-major form so every DMA is a large fully-contiguous
per-partition transfer; all matmuls run fp16 (full-rate PE) with fp32
PSUM accumulation.

Per core:
  - token scan (find '@' span, CITSEG pos) on DVE in f32 math
  - span-masked max-pool over S: per-sample mask-add spread across
    ACT/DVE/GPSIMD, pairwise max (DVE, fp16 2x mode), PE fp16
    transposes + one free-dim max reduce -> pooled^T [768, 32]
  - CITSEG row gather via indirect DMA, proj+enc GEMMs feature-major
    (outputs land pre-transposed for the MLP, no extra transposes)
  - 3-layer MLP batch-major: x stationary, weight column-blocks
    streamed; w2 is host-re-laid-out into column-group-major so each
    506-col group is one contiguous DMA and one PSUM bank
Output [32, 6] f32 per core, concatenated on host to [256, 6].
"""

import sys

for _p in ("/opt/trn_rl_repo", "/root/.axon_site/_ro/trn_rl_repo"):
    if _p not in sys.path:
        sys.path.insert(0, _p)

import numpy as np

# --- problem dims (hardcoded per harness contract) ---
B, S, H = 256, 512, 768
CIT, D1, D2, NCLS = 750, 1518, 3036, 6
NCORES = 8
BPC = B // NCORES  # 32 samples per core
P = 128
AT_ID, CITSEG_ID = 5, 7
NEG = 60000.0  # mask penalty (exactly representable in fp16)
NCH = S // P  # 4 s-chunks of 128
NHT = H // P  # 6 h-tiles
NKX = 12  # ceil(D1/128): 1518 -> 1536
NKH = 24  # ceil(D2/128): 3036 -> 3072
NCT = 6  # ceil(CIT/128): 750 -> 768
GW = 506  # MLP column group (506*4B = 2024 <= one PSUM bank)
NG = D2 // GW  # 6 groups
# bias pack offsets
OB1, OB2, OB3, OPB, OEB = 0, D2, 2 * D2, 2 * D2 + NCLS, 2 * D2 + NCLS + CIT
NBIAS = 2 * D2 + NCLS + 2 * CIT  # 7578

_CACHED = {}


def _build_bass():
    from concourse import bacc, bass, mybir
    import concourse.tile as tile
    from concourse.masks import make_identity

    dt = mybir.dt
    op = mybir.AluOpType
    act = mybir.ActivationFunctionType
    ax = mybir.AxisListType

    f32, i32, f16 = dt.float32, dt.int32, dt.float16

    nc = bacc.Bacc("TRN2", target_bir_lowering=False, debug=False)

    # ---- DRAM parameters (host pre-laid-out, fp16) ----
    tokens_d = nc.declare_dram_parameter("tokens", [BPC, S], i32, isOutput=False)
    hidden_d = nc.declare_dram_parameter("hidden", [P, BPC, NCH, H], f16, isOutput=False)
    projw_d = nc.declare_dram_parameter("proj_w", [P, NHT, CIT], f16, isOutput=False)
    encw_d = nc.declare_dram_parameter("enc_w", [P, NCT, CIT], f16, isOutput=False)
    w1_d = nc.declare_dram_parameter("w1", [P, NG, NKX, GW], f16, isOutput=False)
    w2_d = nc.declare_dram_parameter("w2", [P, NG, NKH, GW], f16, isOutput=False)
    w3_d = nc.declare_dram_parameter("w3", [P, NKH, NCLS], f16, isOutput=False)
    bias_d = nc.declare_dram_parameter("bias", [1, NBIAS], f16, isOutput=False)
    out_d = nc.declare_dram_parameter("out", [BPC, NCLS], f32, isOutput=True)

    with tile.TileContext(nc) as tc:
        with (
            tc.tile_pool(name="consts", bufs=1) as cpool,
            tc.tile_pool(name="hb", bufs=3) as hbp,
            tc.tile_pool(name="mx", bufs=2) as mxp,
            tc.tile_pool(name="wmov", bufs=2) as wp,
            tc.tile_pool(name="pst", bufs=3, space="PSUM") as pst,
            tc.tile_pool(name="psg", bufs=2, space="PSUM") as psg,
        ):
            # ---------- constants ----------
            identh = cpool.tile([P, P], f16, tag="identh")
            make_identity(nc, identh[:])
            ones_col = cpool.tile([1, BPC], f16, tag="ones_col")
            nc.vector.memset(ones_col[:], 1.0)
            ones_row = cpool.tile([1, P], f16, tag="ones_row")
            nc.vector.memset(ones_row[:], 1.0)

            bias_sb = cpool.tile([1, NBIAS], f16, tag="bias_sb")
            nc.sync.dma_start(out=bias_sb[:], in_=bias_d[:])

            # small resident weights
            projw_sb = cpool.tile([P, NHT, CIT], f16, tag="projw_sb")
            nc.sync.dma_start(out=projw_sb[:], in_=projw_d[:])
            encw_sb = cpool.tile([P, NCT, CIT], f16, tag="encw_sb")
            nc.sync.dma_start(out=encw_sb[:], in_=encw_d[:])
            w3_sb = cpool.tile([P, NKH, NCLS], f16, tag="w3_sb")
            nc.sync.dma_start(out=w3_sb[:], in_=w3_d[:])

            # ---------- phase 0: token scan (f32, exact) ----------
            tok_i = cpool.tile([BPC, S], i32, tag="tok_i")
            nc.sync.dma_start(out=tok_i[:], in_=tokens_d[:])
            tok = cpool.tile([BPC, S], f32, tag="tok")
            nc.vector.tensor_copy(out=tok[:], in_=tok_i[:])

            iota_i = cpool.tile([BPC, S], i32, tag="iota_i")
            nc.gpsimd.iota(iota_i[:], pattern=[[1, S]], base=0, channel_multiplier=0)
            iot = cpool.tile([BPC, S], f32, tag="iot")
            nc.vector.tensor_copy(out=iot[:], in_=iota_i[:])

            biota_i = cpool.tile([BPC, 1], i32, tag="biota_i")
            nc.gpsimd.iota(biota_i[:], pattern=[[0, 1]], base=0, channel_multiplier=1)
            biota = cpool.tile([BPC, 1], f32, tag="biota")
            nc.vector.tensor_copy(out=biota[:], in_=biota_i[:])

            def ts_(out_ap, in_ap, s1, o1, s2=None, o2=op.bypass):
                nc.vector.tensor_scalar(
                    out=out_ap, in0=in_ap, scalar1=s1, scalar2=s2, op0=o1, op1=o2
                )

            def tt_(out_ap, a, b_, o):
                nc.vector.tensor_tensor(out=out_ap, in0=a, in1=b_, op=o)

            def bc(ap_, shape):
                return ap_.to_broadcast(shape)

            t_a = cpool.tile([BPC, S], f32, tag="t_a")
            t_b = cpool.tile([BPC, S], f32, tag="t_b")
            t_c = cpool.tile([BPC, S], f32, tag="t_c")
            s_1 = cpool.tile([BPC, 1], f32, tag="s_1")
            s_2 = cpool.tile([BPC, 1], f32, tag="s_2")
            first = cpool.tile([BPC, 1], f32, tag="first")
            second = cpool.tile([BPC, 1], f32, tag="second")
            ge2 = cpool.tile([BPC, 1], f32, tag="ge2")
            start = cpool.tile([BPC, 1], f32, tag="start")
            end = cpool.tile([BPC, 1], f32, tag="end")
            keep = cpool.tile([BPC, S], f32, tag="keep")
            keepany = cpool.tile([BPC, 1], f32, tag="keepany")
            maskneg = cpool.tile([BPC, S], f32, tag="maskneg")
            hasc = cpool.tile([BPC, 1], f32, tag="hasc")
            spos = cpool.tile([BPC, 1], f32, tag="spos")
            cst = cpool.tile([BPC, 1], f32, tag="cst")
            gidx_f = cpool.tile([BPC, 1], f32, tag="gidx_f")
            gidx_i = cpool.tile([BPC, 1], i32, tag="gidx_i")

            # at = (tok == 5); t = at*-1000+1000 + iota; first = min(t)
            ts_(t_c[:], tok[:], float(AT_ID), op.is_equal)
            ts_(t_a[:], t_c[:], -1000.0, op.mult, 1000.0, op.add)
            tt_(t_a[:], t_a[:], iot[:], op.add)
            nc.vector.tensor_reduce(out=first[:], in_=t_a[:], axis=ax.X, op=op.min)
            # second: at & (iota > first)
            tt_(t_b[:], iot[:], bc(first[:], [BPC, S]), op.is_gt)
            tt_(t_b[:], t_b[:], t_c[:], op.mult)
            ts_(t_b[:], t_b[:], -1000.0, op.mult, 1000.0, op.add)
            tt_(t_b[:], t_b[:], iot[:], op.add)
            nc.vector.tensor_reduce(out=second[:], in_=t_b[:], axis=ax.X, op=op.min)
            # ge2 = (sum(at) >= 2)
            nc.vector.tensor_reduce(out=s_1[:], in_=t_c[:], axis=ax.X, op=op.add)
            ts_(ge2[:], s_1[:], 2.0, op.is_ge)
            # start = first*ge2 ; end = (second-512)*ge2 + 512
            tt_(start[:], first[:], ge2[:], op.mult)
            ts_(end[:], second[:], -float(S), op.add)
            tt_(end[:], end[:], ge2[:], op.mult)
            ts_(end[:], end[:], float(S), op.add)
            # keep = (iota < start) | (iota > end)
            tt_(t_a[:], iot[:], bc(start[:], [BPC, S]), op.is_lt)
            tt_(t_b[:], iot[:], bc(end[:], [BPC, S]), op.is_gt)
            tt_(keep[:], t_a[:], t_b[:], op.max)
            nc.vector.tensor_reduce(out=keepany[:], in_=keep[:], axis=ax.X, op=op.max)
            # maskneg = (keep-1)*NEG  -> 0 where keep, -NEG where masked
            ts_(maskneg[:], keep[:], -1.0, op.add, NEG, op.mult)
            # CITSEG first occurrence
            ts_(t_c[:], tok[:], float(CITSEG_ID), op.is_equal)
            ts_(t_a[:], t_c[:], -1000.0, op.mult, 1000.0, op.add)
            tt_(t_a[:], t_a[:], iot[:], op.add)
            nc.vector.tensor_reduce(out=s_1[:], in_=t_a[:], axis=ax.X, op=op.min)
            ts_(hasc[:], s_1[:], float(S - 1), op.is_le)
            ts_(spos[:], s_1[:], float(S - 1), op.min)
            # p-major gather index: s = c*128 + p -> row = p*128 + b*4 + c
            ts_(cst[:], spos[:], 128.0, op.is_ge)
            ts_(s_1[:], spos[:], 256.0, op.is_ge)
            ts_(s_2[:], spos[:], 384.0, op.is_ge)
            tt_(cst[:], cst[:], s_1[:], op.add)
            tt_(cst[:], cst[:], s_2[:], op.add)
            # p = spos - 128*c ; gidx = p*128 + biota*4 + c
            ts_(s_1[:], cst[:], -128.0, op.mult)
            tt_(s_1[:], s_1[:], spos[:], op.add)  # p
            ts_(gidx_f[:], s_1[:], 128.0, op.mult)
            ts_(s_2[:], biota[:], 4.0, op.mult)
            tt_(gidx_f[:], gidx_f[:], s_2[:], op.add)
            tt_(gidx_f[:], gidx_f[:], cst[:], op.add)
            nc.vector.tensor_copy(out=gidx_i[:], in_=gidx_f[:])

            # fp16 casts of per-sample scalars/masks
            maskh = cpool.tile([BPC, S], f16, tag="maskh")
            nc.vector.tensor_copy(out=maskh[:], in_=maskneg[:])
            hasch = cpool.tile([BPC, 1], f16, tag="hasch")
            nc.vector.tensor_copy(out=hasch[:], in_=hasc[:])
            kanyh = cpool.tile([BPC, 1], f16, tag="kanyh")
            nc.vector.tensor_copy(out=kanyh[:], in_=keepany[:])

            # ---------- transposed masks: [128 s, chunk, 32 b] fp16 + f32 ----------
            maskcols_f = cpool.tile([P, NCH, BPC], f32, tag="maskcols_f")
            for c in range(NCH):
                pt = pst.tile([P, NHT, P], f16, tag="pt")
                nc.tensor.transpose(
                    out=pt[:, 0, :BPC], in_=maskh[:, c * P:(c + 1) * P],
                    identity=identh[:BPC, :BPC],
                )
                nc.vector.tensor_copy(out=maskcols_f[:, c, :], in_=pt[:, 0, :BPC])

            hasc_row = cpool.tile([1, BPC], f16, tag="hasc_row")
            pt = pst.tile([P, NHT, P], f16, tag="pt")
            nc.tensor.transpose(
                out=pt[:1, 0, :BPC], in_=hasch[:], identity=identh[:BPC, :BPC]
            )
            nc.vector.tensor_copy(out=hasc_row[:], in_=pt[:1, 0, :BPC])

            ka_row = cpool.tile([1, BPC], f16, tag="ka_row")
            pt = pst.tile([P, NHT, P], f16, tag="pt")
            nc.tensor.transpose(
                out=pt[:1, 0, :BPC], in_=kanyh[:], identity=identh[:BPC, :BPC]
            )
            nc.vector.tensor_copy(out=ka_row[:], in_=pt[:1, 0, :BPC])
            kab = cpool.tile([P, BPC], f16, tag="kab")
            psk = psg.tile([P, GW], f32, tag="pg")
            nc.tensor.matmul(
                out=psk[:, :BPC], lhsT=ones_row[:], rhs=ka_row[:], start=True, stop=True
            )
            nc.vector.tensor_copy(out=kab[:], in_=psk[:, :BPC])

            # ---------- pooling over S (per sample) ----------
            xT = cpool.tile([P, NKX, BPC], f16, tag="xT")
            nc.vector.memset(xT[:], 0.0)
            SPD = 2  # samples per DMA
            for j in range(BPC // SPD):
                hb = hbp.tile([P, SPD, NCH, H], f16, tag="hb")
                nc.sync.dma_start(out=hb[:], in_=hidden_d[:, j * SPD:(j + 1) * SPD, :, :])
                for k in range(SPD):
                    b = j * SPD + k
                    hbb = hb[:, k, :, :]
                    # masked add: chunks 0,1 on ACT; 2 on DVE; 3 on GPSIMD
                    for c in range(2):
                        nc.scalar.activation(
                            out=hbb[:, c, :], in_=hbb[:, c, :],
                            func=act.Identity, bias=maskcols_f[:, c, b:b + 1], scale=1.0,
                        )
                    nc.vector.tensor_scalar(
                        out=hbb[:, 2, :], in0=hbb[:, 2, :],
                        scalar1=maskcols_f[:, 2, b:b + 1], scalar2=None, op0=op.add,
                    )
                    nc.gpsimd.tensor_scalar(
                        out=hbb[:, 3, :], in0=hbb[:, 3, :],
                        scalar1=maskcols_f[:, 3, b:b + 1], scalar2=None, op0=op.add,
                    )
                    mx = mxp.tile([P, 2, H], f16, tag="mx")
                    tt_(mx[:], hbb[:, 0:2, :], hbb[:, 2:4, :], op.max)
                    acc = mxp.tile([P, H], f16, tag="acc")
                    tt_(acc[:], mx[:, 0, :], mx[:, 1, :], op.max)
                    # transpose h-chunks to psum fp16; one free-dim max reduce
                    ptr = pst.tile([P, NHT, P], f16, tag="pt")
                    for t in range(NHT):
                        nc.tensor.transpose(
                            out=ptr[:, t, :], in_=acc[:, t * P:(t + 1) * P],
                            identity=identh[:],
                        )
                    nc.vector.tensor_reduce(
                        out=xT[:, 0:NHT, b], in_=ptr[:], axis=ax.X, op=op.max
                    )
            # zero pooled where no kept position
            for t in range(NHT):
                tt_(xT[:, t, :], xT[:, t, :], kab[:], op.mult)

            # ---------- CITSEG gather + cith^T ----------
            cith = cpool.tile([BPC, H], f16, tag="cith")
            hid_flat = hidden_d[:].rearrange("p b c h -> (p b c) h")
            nc.gpsimd.indirect_dma_start(
                out=cith[:],
                out_offset=None,
                in_=hid_flat,
                in_offset=bass.IndirectOffsetOnAxis(ap=gidx_i[:, :1], axis=0),
            )
            tt_(cith[:], cith[:], bc(hasch[:], [BPC, H]), op.mult)
            cithT = cpool.tile([P, NHT, BPC], f16, tag="cithT")
            for t in range(NHT):
                pt = pst.tile([P, NHT, P], f16, tag="pt")
                nc.tensor.transpose(
                    out=pt[:, 0, :BPC], in_=cith[:, t * P:(t + 1) * P],
                    identity=identh[:BPC, :BPC],
                )
                nc.vector.tensor_copy(out=cithT[:, t, :], in_=pt[:, 0, :BPC])

            # ---------- proj + enc (feature-major; outputs pre-transposed) ----------
            cpT = cpool.tile([P, NCT, BPC], f16, tag="cpT")
            nc.vector.memset(cpT[:], 0.0)
            for mt in range(NCT):
                moff = mt * P
                msz = min(P, CIT - moff)
                ps = psg.tile([P, GW], f32, tag="pg")
                for kt in range(NHT):
                    nc.tensor.matmul(
                        out=ps[:msz, :BPC],
                        lhsT=projw_sb[:, kt, moff:moff + msz],
                        rhs=cithT[:, kt, :],
                        start=(kt == 0), stop=False,
                    )
                nc.tensor.matmul(
                    out=ps[:msz, :BPC],
                    lhsT=bias_sb[:, OPB + moff:OPB + moff + msz],
                    rhs=hasc_row[:], start=False, stop=True,
                )
                nc.vector.tensor_copy(out=cpT[:msz, mt, :], in_=ps[:msz, :BPC])
            for mt in range(NCT):
                moff = mt * P
                msz = min(P, CIT - moff)
                ps = psg.tile([P, GW], f32, tag="pg")
                for kt in range(NCT):
                    nc.tensor.matmul(
                        out=ps[:msz, :BPC],
                        lhsT=encw_sb[:, kt, moff:moff + msz],
                        rhs=cpT[:, kt, :],
                        start=(kt == 0), stop=False,
                    )
                nc.tensor.matmul(
                    out=ps[:msz, :BPC],
                    lhsT=bias_sb[:, OEB + moff:OEB + moff + msz],
                    rhs=ones_col[:], start=False, stop=True,
                )
                nc.vector.tensor_copy(out=xT[:msz, NHT + mt, :], in_=ps[:msz, :BPC])

            # ---------- MLP (batch-major: x stationary, weights moving) ----------
            h1 = cpool.tile([BPC, D2], f16, tag="h1")
            for g in range(NG):
                wc1 = wp.tile([P, NKH, GW], f16, tag="wc")
                nc.sync.dma_start(out=wc1[:, :NKX, :], in_=w1_d[:, g, :, :])
                pg = psg.tile([P, GW], f32, tag="pg")
                for kt in range(NKX):
                    nc.tensor.matmul(
                        out=pg[:BPC, :],
                        lhsT=xT[:, kt, :],
                        rhs=wc1[:, kt, :],
                        start=(kt == 0), stop=False,
                    )
                nc.tensor.matmul(
                    out=pg[:BPC, :], lhsT=ones_col[:],
                    rhs=bias_sb[:, OB1 + g * GW:OB1 + (g + 1) * GW],
                    start=False, stop=True,
                )
                nc.scalar.activation(
                    out=h1[:, g * GW:(g + 1) * GW], in_=pg[:BPC, :], func=act.Relu
                )

            h1T = cpool.tile([P, NKH, BPC], f16, tag="h1T")
            nc.vector.memset(h1T[:], 0.0)
            for t in range(NKH):
                toff = t * P
                tsz = min(P, D2 - toff)
                pt = pst.tile([P, NHT, P], f16, tag="pt")
                nc.tensor.transpose(
                    out=pt[:tsz, 0, :BPC], in_=h1[:, toff:toff + tsz],
                    identity=identh[:BPC, :BPC],
                )
                nc.vector.tensor_copy(out=h1T[:tsz, t, :], in_=pt[:tsz, 0, :BPC])

            h2 = cpool.tile([BPC, D2], f16, tag="h2")
            for g in range(NG):
                wc = wp.tile([P, NKH, GW], f16, tag="wc")
                nc.sync.dma_start(out=wc[:], in_=w2_d[:, g, :, :])
                pg = psg.tile([P, GW], f32, tag="pg")
                for kt in range(NKH):
                    nc.tensor.matmul(
                        out=pg[:BPC, :],
                        lhsT=h1T[:, kt, :],
                        rhs=wc[:, kt, :],
                        start=(kt == 0), stop=False,
                    )
                nc.tensor.matmul(
                    out=pg[:BPC, :], lhsT=ones_col[:],
                    rhs=bias_sb[:, OB2 + g * GW:OB2 + (g + 1) * GW],
                    start=False, stop=True,
                )
                nc.scalar.activation(
                    out=h2[:, g * GW:(g + 1) * GW], in_=pg[:BPC, :], func=act.Relu
                )

            h2T = cpool.tile([P, NKH, BPC], f16, tag="h2T")
            nc.vector.memset(h2T[:], 0.0)
            for t in range(NKH):
                toff = t * P
                tsz = min(P, D2 - toff)
                pt = pst.tile([P, NHT, P], f16, tag="pt")
                nc.tensor.transpose(
                    out=pt[:tsz, 0, :BPC], in_=h2[:, toff:toff + tsz],
                    identity=identh[:BPC, :BPC],
                )
                nc.vector.tensor_copy(out=h2T[:tsz, t, :], in_=pt[:tsz, 0, :BPC])

            po = psg.tile([P, GW], f32, tag="pg")
            for kt in range(NKH):
                nc.tensor.matmul(
                    out=po[:BPC, :NCLS],
                    lhsT=h2T[:, kt, :],
                    rhs=w3_sb[:, kt, :],
                    start=(kt == 0), stop=False,
                )
            nc.tensor.matmul(
                out=po[:BPC, :NCLS], lhsT=ones_col[:],
                rhs=bias_sb[:, OB3:OB3 + NCLS], start=False, stop=True,
            )
            out_sb = cpool.tile([BPC, NCLS], f32, tag="out_sb")
            nc.vector.tensor_copy(out=out_sb[:], in_=po[:BPC, :NCLS])
            nc.sync.dma_start(out=out_d[:], in_=out_sb[:])

    nc.compile()
    return nc


def _get_nc():
    if "nc" not in _CACHED:
        _CACHED["nc"] = _build_bass()
    return _CACHED["nc"]


def _prep_shared(inputs):
    """Host-side fp16 cast + partition-major re-layout of the weights."""
    f16 = np.float16

    def pmajor(w, kpad, label):
        # [K, N] -> zero-pad K to kpad -> [kpad//128, 128, N] -> [128, kt, N]
        K, N = w.shape
        wp = np.zeros((kpad, N), dtype=f16)
        wp[:K] = w.astype(f16)
        return np.ascontiguousarray(
            wp.reshape(kpad // P, P, N).transpose(1, 0, 2)
        )

    # column-group-major: [128, kt, 3036] -> [128, 6, kt, 506]
    w1 = pmajor(np.asarray(inputs["w1"], np.float32), NKX * P, "w1")
    w1 = np.ascontiguousarray(w1.reshape(P, NKX, NG, GW).transpose(0, 2, 1, 3))
    w2 = pmajor(np.asarray(inputs["w2"], np.float32), NKH * P, "w2")
    w2 = np.ascontiguousarray(w2.reshape(P, NKH, NG, GW).transpose(0, 2, 1, 3))
    w3 = pmajor(np.asarray(inputs["w3"], np.float32), NKH * P, "w3")  # [128,24,6]
    pw = pmajor(np.asarray(inputs["proj_w"], np.float32), NHT * P, "pw")  # [128,6,750]
    ew = pmajor(np.asarray(inputs["enc_w"], np.float32), NCT * P, "ew")  # [128,6,750]
    bias = np.concatenate([
        np.asarray(inputs["b1"], np.float32),
        np.asarray(inputs["b2"], np.float32),
        np.asarray(inputs["b3"], np.float32),
        np.asarray(inputs["proj_b"], np.float32),
        np.asarray(inputs["enc_b"], np.float32),
    ]).astype(f16).reshape(1, NBIAS)
    return {
        "w1": w1, "w2": w2, "w3": w3, "proj_w": pw, "enc_w": ew, "bias": bias,
    }


def kernel(**inputs) -> np.ndarray:
    from concourse.bass_utils import run_bass_kernel_spmd

    nc = _get_nc()

    tokens = np.asarray(inputs["tokens"]).astype(np.int32)
    hidden = np.asarray(inputs["hidden_states"], dtype=np.float32).astype(np.float16)
    shared = _prep_shared(inputs)

    in_maps = []
    for i in range(NCORES):
        sl = slice(i * BPC, (i + 1) * BPC)
        m = dict(shared)
        m["tokens"] = np.ascontiguousarray(tokens[sl])
        # [32,512,768] -> s=(c,p) -> [128, 32, 4, 768] partition-major
        hsh = hidden[sl].reshape(BPC, NCH, P, H).transpose(2, 0, 1, 3)
        m["hidden"] = np.ascontiguousarray(hsh)
        in_maps.append(m)

    res = run_bass_kernel_spmd(
        nc, in_maps, core_ids=list(range(NCORES)), trace=bool(_CACHED.get("trace"))
    )
    _CACHED["last_res"] = res
    out = np.concatenate([res.results[i]["out"] for i in range(NCORES)], axis=0)
    return out.astype(np.float32)


if __name__ == "__main__":
    # quick self-test against a numpy reference
    rng = np.random.default_rng(0)
    ins = {
        "tokens": rng.integers(0, 100, (B, S)).astype(np.int64),
        "hidden_states": rng.standard_normal((B, S, H)).astype(np.float32),
        "proj_w": (rng.standard_normal((H, CIT)) / np.sqrt(H)).astype(np.float32),
        "proj_b": (rng.standard_normal(CIT) * 0.02).astype(np.float32),
        "enc_w": (rng.standard_normal((CIT, CIT)) / np.sqrt(CIT)).astype(np.float32),
        "enc_b": (rng.standard_normal(CIT) * 0.02).astype(np.float32),
        "w1": (rng.standard_normal((D1, D2)) / np.sqrt(D1)).astype(np.float32),
        "b1": (rng.standard_normal(D2) * 0.02).astype(np.float32),
        "w2": (rng.standard_normal((D2, D2)) / np.sqrt(D2)).astype(np.float32),
        "b2": (rng.standard_normal(D2) * 0.02).astype(np.float32),
        "w3": (rng.standard_normal((D2, NCLS)) / np.sqrt(D2)).astype(np.float32),
        "b3": (rng.standard_normal(NCLS) * 0.02).astype(np.float32),
    }
    got = kernel(**ins)
    print("kernel out", got.shape, got.dtype, got[:2])


# revision 21
# speedup vs baseline: 2.4299x; 2.4299x over previous
"""Trainium2 Bass kernel for nn_CitationClassifier (pooling/ridge).

Strategy: pure data parallel over the batch dim (256 = 8 cores x 32).
All heavy tensors are cast to fp16 and re-laid-out on the host into
partition-major form so every DMA is a large fully-contiguous
per-partition transfer; all matmuls run fp16 (full-rate PE) with fp32
PSUM accumulation.

Per core:
  - token scan (find '@' span, CITSEG pos) on DVE in f32 math
  - span-masked max-pool over S: per-sample mask-add spread across
    ACT/DVE/GPSIMD, pairwise max (DVE, fp16 2x mode), PE fp16
    transposes + one free-dim max reduce -> pooled^T [768, 32]
  - CITSEG row gather via indirect DMA, proj+enc GEMMs feature-major
    (outputs land pre-transposed for the MLP, no extra transposes)
  - 3-layer MLP batch-major: x stationary, weight column-blocks
    streamed; w2 is host-re-laid-out into column-group-major so each
    506-col group is one contiguous DMA and one PSUM bank
Output [32, 6] f32 per core, concatenated on host to [256, 6].
"""

import sys

for _p in ("/opt/trn_rl_repo", "/root/.axon_site/_ro/trn_rl_repo"):
    if _p not in sys.path:
        sys.path.insert(0, _p)

import numpy as np

# --- problem dims (hardcoded per harness contract) ---
B, S, H = 256, 512, 768
CIT, D1, D2, NCLS = 750, 1518, 3036, 6
NCORES = 8
BPC = B // NCORES  # 32 samples per core
P = 128
AT_ID, CITSEG_ID = 5, 7
NEG = 60000.0  # mask penalty (exactly representable in fp16)
NCH = S // P  # 4 s-chunks of 128
NHT = H // P  # 6 h-tiles
NKX = 12  # ceil(D1/128): 1518 -> 1536
NKH = 24  # ceil(D2/128): 3036 -> 3072
NCT = 6  # ceil(CIT/128): 750 -> 768
GW = 506  # MLP column group (506*4B = 2024 <= one PSUM bank)
NG = D2 // GW  # 6 groups
# bias pack offsets
OB1, OB2, OB3, OPB, OEB = 0, D2, 2 * D2, 2 * D2 + NCLS, 2 * D2 + NCLS + CIT
NBIAS = 2 * D2 + NCLS + 2 * CIT  # 7578

_CACHED = {}


def _build_bass():
    from concourse import bacc, bass, mybir
    import concourse.tile as tile
    from concourse.masks import make_identity

    dt = mybir.dt
    op = mybir.AluOpType
    act = mybir.ActivationFunctionType
    ax = mybir.AxisListType

    f32, i32, f16, bf16 = dt.float32, dt.int32, dt.float16, dt.bfloat16

    nc = bacc.Bacc("TRN2", target_bir_lowering=False, debug=False)

    # ---- DRAM parameters (host pre-laid-out, fp16) ----
    tokens_d = nc.declare_dram_parameter("tokens", [BPC, S], i32, isOutput=False)
    hidden_d = nc.declare_dram_parameter("hidden", [P, BPC, NCH, H], bf16, isOutput=False)
    projw_d = nc.declare_dram_parameter("proj_w", [P, NHT, CIT], f16, isOutput=False)
    encw_d = nc.declare_dram_parameter("enc_w", [P, NCT, CIT], f16, isOutput=False)
    w1_d = nc.declare_dram_parameter("w1", [P, NG, NKX, GW], f16, isOutput=False)
    w2_d = nc.declare_dram_parameter("w2", [P, NG, NKH, GW], f16, isOutput=False)
    w3_d = nc.declare_dram_parameter("w3", [P, NKH, NCLS], f16, isOutput=False)
    bias_d = nc.declare_dram_parameter("bias", [1, NBIAS], f16, isOutput=False)
    out_d = nc.declare_dram_parameter("out", [BPC, NCLS], f32, isOutput=True)

    with tile.TileContext(nc) as tc:
        with (
            tc.tile_pool(name="consts", bufs=1) as cpool,
            tc.tile_pool(name="hb", bufs=3) as hbp,
            tc.tile_pool(name="mx", bufs=2) as mxp,
            tc.tile_pool(name="wmov", bufs=2) as wp,
            tc.tile_pool(name="pst", bufs=3, space="PSUM") as pst,
            tc.tile_pool(name="psg", bufs=2, space="PSUM") as psg,
        ):
            # ---------- constants ----------
            identh = cpool.tile([P, P], f16, tag="identh")
            make_identity(nc, identh[:])
            identb = cpool.tile([P, P], bf16, tag="identb")
            make_identity(nc, identb[:])
            ones_col = cpool.tile([1, BPC], f16, tag="ones_col")
            nc.vector.memset(ones_col[:], 1.0)
            ones_row = cpool.tile([1, P], f16, tag="ones_row")
            nc.vector.memset(ones_row[:], 1.0)

            bias_sb = cpool.tile([1, NBIAS], f16, tag="bias_sb")
            nc.sync.dma_start(out=bias_sb[:], in_=bias_d[:])

            # small resident weights
            projw_sb = cpool.tile([P, NHT, CIT], f16, tag="projw_sb")
            nc.sync.dma_start(out=projw_sb[:], in_=projw_d[:])
            encw_sb = cpool.tile([P, NCT, CIT], f16, tag="encw_sb")
            nc.sync.dma_start(out=encw_sb[:], in_=encw_d[:])
            w3_sb = cpool.tile([P, NKH, NCLS], f16, tag="w3_sb")
            nc.sync.dma_start(out=w3_sb[:], in_=w3_d[:])

            # ---------- phase 0: token scan (f32, exact) ----------
            tok_i = cpool.tile([BPC, S], i32, tag="tok_i")
            nc.sync.dma_start(out=tok_i[:], in_=tokens_d[:])
            tok = cpool.tile([BPC, S], f32, tag="tok")
            nc.vector.tensor_copy(out=tok[:], in_=tok_i[:])

            iota_i = cpool.tile([BPC, S], i32, tag="iota_i")
            nc.gpsimd.iota(iota_i[:], pattern=[[1, S]], base=0, channel_multiplier=0)
            iot = cpool.tile([BPC, S], f32, tag="iot")
            nc.vector.tensor_copy(out=iot[:], in_=iota_i[:])

            biota_i = cpool.tile([BPC, 1], i32, tag="biota_i")
            nc.gpsimd.iota(biota_i[:], pattern=[[0, 1]], base=0, channel_multiplier=1)
            biota = cpool.tile([BPC, 1], f32, tag="biota")
            nc.vector.tensor_copy(out=biota[:], in_=biota_i[:])

            def ts_(out_ap, in_ap, s1, o1, s2=None, o2=op.bypass):
                nc.vector.tensor_scalar(
                    out=out_ap, in0=in_ap, scalar1=s1, scalar2=s2, op0=o1, op1=o2
                )

            def tt_(out_ap, a, b_, o):
                nc.vector.tensor_tensor(out=out_ap, in0=a, in1=b_, op=o)

            def bc(ap_, shape):
                return ap_.to_broadcast(shape)

            t_a = cpool.tile([BPC, S], f32, tag="t_a")
            t_b = cpool.tile([BPC, S], f32, tag="t_b")
            t_c = cpool.tile([BPC, S], f32, tag="t_c")
            s_1 = cpool.tile([BPC, 1], f32, tag="s_1")
            s_2 = cpool.tile([BPC, 1], f32, tag="s_2")
            first = cpool.tile([BPC, 1], f32, tag="first")
            second = cpool.tile([BPC, 1], f32, tag="second")
            ge2 = cpool.tile([BPC, 1], f32, tag="ge2")
            start = cpool.tile([BPC, 1], f32, tag="start")
            end = cpool.tile([BPC, 1], f32, tag="end")
            keep = cpool.tile([BPC, S], f32, tag="keep")
            keepany = cpool.tile([BPC, 1], f32, tag="keepany")
            maskneg = cpool.tile([BPC, S], f32, tag="maskneg")
            hasc = cpool.tile([BPC, 1], f32, tag="hasc")
            spos = cpool.tile([BPC, 1], f32, tag="spos")
            cst = cpool.tile([BPC, 1], f32, tag="cst")
            gidx_f = cpool.tile([BPC, 1], f32, tag="gidx_f")
            gidx_i = cpool.tile([BPC, 1], i32, tag="gidx_i")

            # at = (tok == 5); t = at*-1000+1000 + iota; first = min(t)
            ts_(t_c[:], tok[:], float(AT_ID), op.is_equal)
            ts_(t_a[:], t_c[:], -1000.0, op.mult, 1000.0, op.add)
            tt_(t_a[:], t_a[:], iot[:], op.add)
            nc.vector.tensor_reduce(out=first[:], in_=t_a[:], axis=ax.X, op=op.min)
            # second: at & (iota > first)
            tt_(t_b[:], iot[:], bc(first[:], [BPC, S]), op.is_gt)
            tt_(t_b[:], t_b[:], t_c[:], op.mult)
            ts_(t_b[:], t_b[:], -1000.0, op.mult, 1000.0, op.add)
            tt_(t_b[:], t_b[:], iot[:], op.add)
            nc.vector.tensor_reduce(out=second[:], in_=t_b[:], axis=ax.X, op=op.min)
            # ge2 = (sum(at) >= 2)
            nc.vector.tensor_reduce(out=s_1[:], in_=t_c[:], axis=ax.X, op=op.add)
            ts_(ge2[:], s_1[:], 2.0, op.is_ge)
            # start = first*ge2 ; end = (second-512)*ge2 + 512
            tt_(start[:], first[:], ge2[:], op.mult)
            ts_(end[:], second[:], -float(S), op.add)
            tt_(end[:], end[:], ge2[:], op.mult)
            ts_(end[:], end[:], float(S), op.add)
            # keep = (iota < start) | (iota > end)
            tt_(t_a[:], iot[:], bc(start[:], [BPC, S]), op.is_lt)
            tt_(t_b[:], iot[:], bc(end[:], [BPC, S]), op.is_gt)
            tt_(keep[:], t_a[:], t_b[:], op.max)
            nc.vector.tensor_reduce(out=keepany[:], in_=keep[:], axis=ax.X, op=op.max)
            # maskneg = (keep-1)*NEG  -> 0 where keep, -NEG where masked
            ts_(maskneg[:], keep[:], -1.0, op.add, NEG, op.mult)
            # CITSEG first occurrence
            ts_(t_c[:], tok[:], float(CITSEG_ID), op.is_equal)
            ts_(t_a[:], t_c[:], -1000.0, op.mult, 1000.0, op.add)
            tt_(t_a[:], t_a[:], iot[:], op.add)
            nc.vector.tensor_reduce(out=s_1[:], in_=t_a[:], axis=ax.X, op=op.min)
            ts_(hasc[:], s_1[:], float(S - 1), op.is_le)
            ts_(spos[:], s_1[:], float(S - 1), op.min)
            # p-major gather index: s = c*128 + p -> row = p*128 + b*4 + c
            ts_(cst[:], spos[:], 128.0, op.is_ge)
            ts_(s_1[:], spos[:], 256.0, op.is_ge)
            ts_(s_2[:], spos[:], 384.0, op.is_ge)
            tt_(cst[:], cst[:], s_1[:], op.add)
            tt_(cst[:], cst[:], s_2[:], op.add)
            # p = spos - 128*c ; gidx = p*128 + biota*4 + c
            ts_(s_1[:], cst[:], -128.0, op.mult)
            tt_(s_1[:], s_1[:], spos[:], op.add)  # p
            ts_(gidx_f[:], s_1[:], 128.0, op.mult)
            ts_(s_2[:], biota[:], 4.0, op.mult)
            tt_(gidx_f[:], gidx_f[:], s_2[:], op.add)
            tt_(gidx_f[:], gidx_f[:], cst[:], op.add)
            nc.vector.tensor_copy(out=gidx_i[:], in_=gidx_f[:])

            # low-precision casts of per-sample scalars/masks
            maskh = cpool.tile([BPC, S], bf16, tag="maskh")
            nc.vector.tensor_copy(out=maskh[:], in_=maskneg[:])
            hasch = cpool.tile([BPC, 1], f16, tag="hasch")
            nc.vector.tensor_copy(out=hasch[:], in_=hasc[:])
            haschb = cpool.tile([BPC, 1], bf16, tag="haschb")
            nc.vector.tensor_copy(out=haschb[:], in_=hasc[:])
            kanyh = cpool.tile([BPC, 1], f16, tag="kanyh")
            nc.vector.tensor_copy(out=kanyh[:], in_=keepany[:])

            # ---------- transposed masks: [128 s, chunk, 32 b] fp16 + f32 ----------
            maskcols_f = cpool.tile([P, NCH, BPC], f32, tag="maskcols_f")
            for c in range(NCH):
                pt = pst.tile([P, NHT, P], bf16, tag="pt")
                nc.tensor.transpose(
                    out=pt[:, 0, :BPC], in_=maskh[:, c * P:(c + 1) * P],
                    identity=identb[:BPC, :BPC],
                )
                nc.vector.tensor_copy(out=maskcols_f[:, c, :], in_=pt[:, 0, :BPC])

            hasc_row = cpool.tile([1, BPC], f16, tag="hasc_row")
            pt = pst.tile([P, NHT, P], f16, tag="pt")
            nc.tensor.transpose(
                out=pt[:1, 0, :BPC], in_=hasch[:], identity=identh[:BPC, :BPC]
            )
            nc.vector.tensor_copy(out=hasc_row[:], in_=pt[:1, 0, :BPC])

            ka_row = cpool.tile([1, BPC], f16, tag="ka_row")
            pt = pst.tile([P, NHT, P], f16, tag="pt")
            nc.tensor.transpose(
                out=pt[:1, 0, :BPC], in_=kanyh[:], identity=identh[:BPC, :BPC]
            )
            nc.vector.tensor_copy(out=ka_row[:], in_=pt[:1, 0, :BPC])
            kab = cpool.tile([P, BPC], f16, tag="kab")
            psk = psg.tile([P, GW], f32, tag="pg")
            nc.tensor.matmul(
                out=psk[:, :BPC], lhsT=ones_row[:], rhs=ka_row[:], start=True, stop=True
            )
            nc.vector.tensor_copy(out=kab[:], in_=psk[:, :BPC])

            # ---------- pooling over S (per sample) ----------
            xT = cpool.tile([P, NKX, BPC], f16, tag="xT")
            nc.vector.memset(xT[:], 0.0)
            SPD = 2  # samples per DMA
            for j in range(BPC // SPD):
                hb = hbp.tile([P, SPD, NCH, H], bf16, tag="hb")
                nc.sync.dma_start(out=hb[:], in_=hidden_d[:, j * SPD:(j + 1) * SPD, :, :])
                for k in range(SPD):
                    b = j * SPD + k
                    hbb = hb[:, k, :, :]
                    # masked add: chunks 0-2 on ACT; 3 on DVE (bf16 fast path)
                    for c in range(3):
                        nc.scalar.activation(
                            out=hbb[:, c, :], in_=hbb[:, c, :],
                            func=act.Identity, bias=maskcols_f[:, c, b:b + 1], scale=1.0,
                        )
                    nc.vector.tensor_scalar(
                        out=hbb[:, 3, :], in0=hbb[:, 3, :],
                        scalar1=maskcols_f[:, 3, b:b + 1], scalar2=None, op0=op.add,
                    )
                    mx = mxp.tile([P, 2, H], bf16, tag="mx")
                    tt_(mx[:], hbb[:, 0:2, :], hbb[:, 2:4, :], op.max)
                    acc = mxp.tile([P, H], bf16, tag="acc")
                    tt_(acc[:], mx[:, 0, :], mx[:, 1, :], op.max)
                    # transpose h-chunks to psum bf16; one free-dim max reduce
                    ptr = pst.tile([P, NHT, P], bf16, tag="pt")
                    for t in range(NHT):
                        nc.tensor.transpose(
                            out=ptr[:, t, :], in_=acc[:, t * P:(t + 1) * P],
                            identity=identb[:],
                        )
                    nc.vector.tensor_reduce(
                        out=xT[:, 0:NHT, b], in_=ptr[:], axis=ax.X, op=op.max
                    )
            # zero pooled where no kept position
            for t in range(NHT):
                tt_(xT[:, t, :], xT[:, t, :], kab[:], op.mult)

            # ---------- CITSEG gather + cith^T ----------
            cith = cpool.tile([BPC, H], bf16, tag="cith")
            hid_flat = hidden_d[:].rearrange("p b c h -> (p b c) h")
            nc.gpsimd.indirect_dma_start(
                out=cith[:],
                out_offset=None,
                in_=hid_flat,
                in_offset=bass.IndirectOffsetOnAxis(ap=gidx_i[:, :1], axis=0),
            )
            cith16 = cpool.tile([BPC, H], f16, tag="cith16")
            tt_(cith16[:], cith[:], bc(haschb[:], [BPC, H]), op.mult)
            cithT = cpool.tile([P, NHT, BPC], f16, tag="cithT")
            for t in range(NHT):
                pt = pst.tile([P, NHT, P], f16, tag="pt")
                nc.tensor.transpose(
                    out=pt[:, 0, :BPC], in_=cith16[:, t * P:(t + 1) * P],
                    identity=identh[:BPC, :BPC],
                )
                nc.vector.tensor_copy(out=cithT[:, t, :], in_=pt[:, 0, :BPC])

            # ---------- proj + enc (feature-major; outputs pre-transposed) ----------
            cpT = cpool.tile([P, NCT, BPC], f16, tag="cpT")
            nc.vector.memset(cpT[:], 0.0)
            for mt in range(NCT):
                moff = mt * P
                msz = min(P, CIT - moff)
                ps = psg.tile([P, GW], f32, tag="pg")
                for kt in range(NHT):
                    nc.tensor.matmul(
                        out=ps[:msz, :BPC],
                        lhsT=projw_sb[:, kt, moff:moff + msz],
                        rhs=cithT[:, kt, :],
                        start=(kt == 0), stop=False,
                    )
                nc.tensor.matmul(
                    out=ps[:msz, :BPC],
                    lhsT=bias_sb[:, OPB + moff:OPB + moff + msz],
                    rhs=hasc_row[:], start=False, stop=True,
                )
                nc.vector.tensor_copy(out=cpT[:msz, mt, :], in_=ps[:msz, :BPC])
            for mt in range(NCT):
                moff = mt * P
                msz = min(P, CIT - moff)
                ps = psg.tile([P, GW], f32, tag="pg")
                for kt in range(NCT):
                    nc.tensor.matmul(
                        out=ps[:msz, :BPC],
                        lhsT=encw_sb[:, kt, moff:moff + msz],
                        rhs=cpT[:, kt, :],
                        start=(kt == 0), stop=False,
                    )
                nc.tensor.matmul(
                    out=ps[:msz, :BPC],
                    lhsT=bias_sb[:, OEB + moff:OEB + moff + msz],
                    rhs=ones_col[:], start=False, stop=True,
                )
                nc.vector.tensor_copy(out=xT[:msz, NHT + mt, :], in_=ps[:msz, :BPC])

            # ---------- MLP (batch-major: x stationary, weights moving) ----------
            h1 = cpool.tile([BPC, D2], f16, tag="h1")
            for g in range(NG):
                wc1 = wp.tile([P, NKH, GW], f16, tag="wc")
                nc.sync.dma_start(out=wc1[:, :NKX, :], in_=w1_d[:, g, :, :])
                pg = psg.tile([P, GW], f32, tag="pg")
                for kt in range(NKX):
                    nc.tensor.matmul(
                        out=pg[:BPC, :],
                        lhsT=xT[:, kt, :],
                        rhs=wc1[:, kt, :],
                        start=(kt == 0), stop=False,
                    )
                nc.tensor.matmul(
                    out=pg[:BPC, :], lhsT=ones_col[:],
                    rhs=bias_sb[:, OB1 + g * GW:OB1 + (g + 1) * GW],
                    start=False, stop=True,
                )
                nc.scalar.activation(
                    out=h1[:, g * GW:(g + 1) * GW], in_=pg[:BPC, :], func=act.Relu
                )

            h1T = cpool.tile([P, NKH, BPC], f16, tag="h1T")
            nc.vector.memset(h1T[:], 0.0)
            for t in range(NKH):
                toff = t * P
                tsz = min(P, D2 - toff)
                pt = pst.tile([P, NHT, P], f16, tag="pt")
                nc.tensor.transpose(
                    out=pt[:tsz, 0, :BPC], in_=h1[:, toff:toff + tsz],
                    identity=identh[:BPC, :BPC],
                )
                nc.vector.tensor_copy(out=h1T[:tsz, t, :], in_=pt[:tsz, 0, :BPC])

            h2 = cpool.tile([BPC, D2], f16, tag="h2")
            for g in range(NG):
                wc = wp.tile([P, NKH, GW], f16, tag="wc")
                nc.sync.dma_start(out=wc[:], in_=w2_d[:, g, :, :])
                pg = psg.tile([P, GW], f32, tag="pg")
                for kt in range(NKH):
                    nc.tensor.matmul(
                        out=pg[:BPC, :],
                        lhsT=h1T[:, kt, :],
                        rhs=wc[:, kt, :],
                        start=(kt == 0), stop=False,
                    )
                nc.tensor.matmul(
                    out=pg[:BPC, :], lhsT=ones_col[:],
                    rhs=bias_sb[:, OB2 + g * GW:OB2 + (g + 1) * GW],
                    start=False, stop=True,
                )
                nc.scalar.activation(
                    out=h2[:, g * GW:(g + 1) * GW], in_=pg[:BPC, :], func=act.Relu
                )

            h2T = cpool.tile([P, NKH, BPC], f16, tag="h2T")
            nc.vector.memset(h2T[:], 0.0)
            for t in range(NKH):
                toff = t * P
                tsz = min(P, D2 - toff)
                pt = pst.tile([P, NHT, P], f16, tag="pt")
                nc.tensor.transpose(
                    out=pt[:tsz, 0, :BPC], in_=h2[:, toff:toff + tsz],
                    identity=identh[:BPC, :BPC],
                )
                nc.vector.tensor_copy(out=h2T[:tsz, t, :], in_=pt[:tsz, 0, :BPC])

            po = psg.tile([P, GW], f32, tag="pg")
            for kt in range(NKH):
                nc.tensor.matmul(
                    out=po[:BPC, :NCLS],
                    lhsT=h2T[:, kt, :],
                    rhs=w3_sb[:, kt, :],
                    start=(kt == 0), stop=False,
                )
            nc.tensor.matmul(
                out=po[:BPC, :NCLS], lhsT=ones_col[:],
                rhs=bias_sb[:, OB3:OB3 + NCLS], start=False, stop=True,
            )
            out_sb = cpool.tile([BPC, NCLS], f32, tag="out_sb")
            nc.vector.tensor_copy(out=out_sb[:], in_=po[:BPC, :NCLS])
            nc.sync.dma_start(out=out_d[:], in_=out_sb[:])

    nc.compile()
    return nc


def _get_nc():
    if "nc" not in _CACHED:
        _CACHED["nc"] = _build_bass()
    return _CACHED["nc"]


def _prep_shared(inputs):
    """Host-side fp16 cast + partition-major re-layout of the weights."""
    f16 = np.float16

    def pmajor(w, kpad, label):
        # [K, N] -> zero-pad K to kpad -> [kpad//128, 128, N] -> [128, kt, N]
        K, N = w.shape
        wp = np.zeros((kpad, N), dtype=f16)
        wp[:K] = w.astype(f16)
        return np.ascontiguousarray(
            wp.reshape(kpad // P, P, N).transpose(1, 0, 2)
        )

    # column-group-major: [128, kt, 3036] -> [128, 6, kt, 506]
    w1 = pmajor(np.asarray(inputs["w1"], np.float32), NKX * P, "w1")
    w1 = np.ascontiguousarray(w1.reshape(P, NKX, NG, GW).transpose(0, 2, 1, 3))
    w2 = pmajor(np.asarray(inputs["w2"], np.float32), NKH * P, "w2")
    w2 = np.ascontiguousarray(w2.reshape(P, NKH, NG, GW).transpose(0, 2, 1, 3))
    w3 = pmajor(np.asarray(inputs["w3"], np.float32), NKH * P, "w3")  # [128,24,6]
    pw = pmajor(np.asarray(inputs["proj_w"], np.float32), NHT * P, "pw")  # [128,6,750]
    ew = pmajor(np.asarray(inputs["enc_w"], np.float32), NCT * P, "ew")  # [128,6,750]
    bias = np.concatenate([
        np.asarray(inputs["b1"], np.float32),
        np.asarray(inputs["b2"], np.float32),
        np.asarray(inputs["b3"], np.float32),
        np.asarray(inputs["proj_b"], np.float32),
        np.asarray(inputs["enc_b"], np.float32),
    ]).astype(f16).reshape(1, NBIAS)
    return {
        "w1": w1, "w2": w2, "w3": w3, "proj_w": pw, "enc_w": ew, "bias": bias,
    }


def kernel(**inputs) -> np.ndarray:
    from concourse.bass_utils import run_bass_kernel_spmd

    nc = _get_nc()

    import ml_dtypes

    tokens = np.asarray(inputs["tokens"]).astype(np.int32)
    hidden = np.asarray(inputs["hidden_states"], dtype=np.float32).astype(
        ml_dtypes.bfloat16
    )
    shared = _prep_shared(inputs)

    in_maps = []
    for i in range(NCORES):
        sl = slice(i * BPC, (i + 1) * BPC)
        m = dict(shared)
        m["tokens"] = np.ascontiguousarray(tokens[sl])
        # [32,512,768] -> s=(c,p) -> [128, 32, 4, 768] partition-major
        hsh = hidden[sl].reshape(BPC, NCH, P, H).transpose(2, 0, 1, 3)
        m["hidden"] = np.ascontiguousarray(hsh)
        in_maps.append(m)

    res = run_bass_kernel_spmd(
        nc, in_maps, core_ids=list(range(NCORES)), trace=bool(_CACHED.get("trace"))
    )
    _CACHED["last_res"] = res
    out = np.concatenate([res.results[i]["out"] for i in range(NCORES)], axis=0)
    return out.astype(np.float32)


if __name__ == "__main__":
    # quick self-test against a numpy reference
    rng = np.random.default_rng(0)
    ins = {
        "tokens": rng.integers(0, 100, (B, S)).astype(np.int64),
        "hidden_states": rng.standard_normal((B, S, H)).astype(np.float32),
        "proj_w": (rng.standard_normal((H, CIT)) / np.sqrt(H)).astype(np.float32),
        "proj_b": (rng.standard_normal(CIT) * 0.02).astype(np.float32),
        "enc_w": (rng.standard_normal((CIT, CIT)) / np.sqrt(CIT)).astype(np.float32),
        "enc_b": (rng.standard_normal(CIT) * 0.02).astype(np.float32),
        "w1": (rng.standard_normal((D1, D2)) / np.sqrt(D1)).astype(np.float32),
        "b1": (rng.standard_normal(D2) * 0.02).astype(np.float32),
        "w2": (rng.standard_normal((D2, D2)) / np.sqrt(D2)).astype(np.float32),
        "b2": (rng.standard_normal(D2) * 0.02).astype(np.float32),
        "w3": (rng.standard_normal((D2, NCLS)) / np.sqrt(D2)).astype(np.float32),
        "b3": (rng.standard_normal(NCLS) * 0.02).astype(np.float32),
    }
    got = kernel(**ins)
    print("kernel out", got.shape, got.dtype, got[:2])
